# revision 1
# baseline (speedup 1.0000x reference)
"""Trainium2 Bass kernel for nn_CMAAA_29274497089816 (sparse local attention).

Sharding: data-parallel B(2) x H-slab(4) over 8 cores. Each core handles one
batch sample and a 64-row output slab. Host prepares padded input slabs,
folded conv weights (cond/s and pan-lpan folds baked in), and the scrambled
k_ms "S" field (small band conv in numpy); the chip runs the big convs and
the full neighborhood attention.
"""
import sys, os
sys.path.insert(0, "/opt/trn_rl_repo")
import numpy as np
import ml_dtypes

import concourse.bass as bass
import concourse.bacc as bacc
import concourse.mybir as mybir
from concourse import tile
from concourse.bass_utils import run_bass_kernel_spmd

BF16 = mybir.dt.bfloat16
F32 = mybir.dt.float32
AF = mybir.ActivationFunctionType
ALU = mybir.AluOpType

DIM, HEADS, KA, MS_C, B, H, W = 32, 8, 3, 8, 2, 256, 256
HD, KK = 4, 9
SCALE = HD ** -0.5

NROW = 66            # field rows r0-1 .. r1+1
WP = 258             # padded width
NF = NROW * WP       # 17028 field pixels
FM = 2               # front/back margin elems in field tiles
NBLK = 4             # attention row-blocks per core
BR = 16              # out rows per block
PGRID = BR * WP      # 4128 real product px per block
NCH = 9              # chunks per block (9*512 = 4608 >= 4128)
CH = 512
PF = NCH * CH        # 4608 padded product px
RMARG = 2 * WP + 2   # replica tile read margin
RLEN = 20 * WP + 8


def _np(x):
    return np.ascontiguousarray(x)


# ---------------------------------------------------------------- host prep
def _fold_main_weights(w_q, w_kvms, w_vpan, sb):
    """lhsT_main[9, 51, 128]: channels [x32, msQ8, lpanQ1, ms8, lpan1, pan1],
    outputs [q(scaled)32, k_ms32, v_ms32, v_pan32]."""
    Ls = np.zeros((9, 51, 128), np.float32)
    i = 0
    for dy in range(3):
        for dx in range(3):
            L = Ls[i]; i += 1
            Wq = w_q[:, :, dy, dx]
            L[0:32, 0:32] = Wq[:, 0:32].T * SCALE
            L[32:40, 0:32] = Wq[:, 32:40].T * SCALE * sb
            L[40, 0:32] = Wq[:, 32:40].sum(1) * SCALE * (1.0 - sb)
            Wk = w_kvms[:, :, dy, dx]
            L[0:32, 32:64] = Wk[0:32, 0:32].T
            L[41:49, 32:64] = Wk[0:32, 32:40].T
            L[0:32, 64:96] = Wk[32:64, 0:32].T
            L[41:49, 64:96] = Wk[32:64, 32:40].T
            Wv = w_vpan[:, :, dy, dx]
            L[0:32, 96:128] = Wv[:, 0:32].T
            L[49, 96:128] = Wv[:, 32] - Wv[:, 34]
            L[50, 96:128] = Wv[:, 33] + Wv[:, 34]
    return Ls


def _attn_weights(w_dep, b_dep, w_proj_pan, b_proj_pan, w_proj_ms, b_proj_ms):
    Wd = np.zeros((4, 9, 9), np.float32)          # [d, t, j]
    for d in range(4):
        for j in range(9):
            Wd[d, :, j] = w_dep[d * 9 + j, 0].reshape(9)
    bd = b_dep.reshape(4, 9)                      # [d, j]

    # logits MM weights: lhsT_L[dy] [128, 72]; rows (dx,h,d) 0:96, q-rows 96:128
    L_L = np.zeros((3, 128, 72), np.float32)
    for dy in range(3):
        for dx in range(3):
            t = dy * 3 + dx
            for h in range(8):
                for d in range(4):
                    for j in range(9):
                        L_L[dy, dx * 32 + h * 4 + d, h * 9 + j] = Wd[d, t, j]
    for h in range(8):
        for d in range(4):
            for j in range(9):
                L_L[1, 96 + h * 4 + d, h * 9 + j] = bd[d, j]   # qb bias term

    # s0 sum MM: lhsT_s [72, 8]
    L_s = np.zeros((72, 8), np.float32)
    for h in range(8):
        L_s[h * 9:(h + 1) * 9, h] = 1.0
    # R72 broadcast MM: lhsT_R [8, 72]
    L_R = np.zeros((8, 72), np.float32)
    for h in range(8):
        L_R[h, h * 9:(h + 1) * 9] = 1.0
    # A MMs: lhsT_A[dy] [72, 128]: cols (dx,h,d) 0:96; dy==1 cols 96:128 = ba
    L_A = np.zeros((3, 72, 128), np.float32)
    for dy in range(3):
        for dx in range(3):
            t = dy * 3 + dx
            for h in range(8):
                for d in range(4):
                    for j in range(9):
                        L_A[dy, h * 9 + j, dx * 32 + h * 4 + d] = Wd[d, t, j]
    for h in range(8):
        for d in range(4):
            for j in range(9):
                L_A[1, h * 9 + j, 96 + h * 4 + d] = bd[d, j]
    # proj: lhsT_P[2, 128, 32]: rows (dx,h,d) = Wp.T replicated; rows 96:128 Wp.T
    L_P = np.zeros((2, 128, 32), np.float32)
    for bi, wp in enumerate([w_proj_pan, w_proj_ms]):
        wt = wp[:, :, 0, 0].T                     # [32in(h,d), 32out]
        for dx in range(3):
            L_P[bi, dx * 32:(dx + 1) * 32] = wt
        L_P[bi, 96:128] = wt
    pbias = np.stack([b_proj_pan, b_proj_ms]).reshape(2, 32, 1).astype(np.float32)
    return L_L, L_s, L_R, L_A, L_P, pbias


def _host_sfield(x, ms, w_kvms, b, r0):
    """Scrambled k_ms field [32,(h,d')], rows r0-1..r1+1, via numpy band conv."""
    R1 = r0 + 64
    need = {}
    for X in range(r0 - 1, R1 + 1):
        if 0 <= X < 256:
            need.setdefault(X % 4, set()).update(
                {64 * dp + X // 4 for dp in range(4)})
    cols = sorted(set().union(*need.values()))
    # conv inputs at cols +-1, all rows, zero padded
    xin = np.concatenate([x[b], ms[b]], 0)        # (40, 256, 256)
    xp = np.pad(xin, ((0, 0), (1, 1), (1, 1)))
    Wk = w_kvms[0:32]                             # k half (32, 40, 3, 3)
    kcols = np.zeros((32, 256, 256), np.float32)  # only needed cols filled
    for c in cols:
        acc = np.zeros((32, 256), np.float32)
        for dy in range(3):
            for dx in range(3):
                acc += np.einsum("oc,cy->oy", Wk[:, :, dy, dx],
                                 xp[:, dy:dy + 256, c + dx])
        kcols[:, :, c] = acc
    S = np.zeros((32, NROW, WP), np.float32)
    for hh in range(8):
        for dp in range(4):
            for gi, X in enumerate(range(r0 - 1, R1 + 1)):
                if 0 <= X < 256:
                    S[hh * 4 + dp, gi, 1:257] = \
                        kcols[hh * 4 + (X % 4), :, 64 * dp + X // 4]
    return S


# ---------------------------------------------------------------- bass build
_CACHE = {}


def _build_nc():
    if "nc" in _CACHE:
        return _CACHE["nc"]
    nc = bacc.Bacc(None, target_bir_lowering=False)
    FDL = 2 + NF + 524
    xin_d = nc.declare_dram_parameter("xin", [51, 68 * WP], BF16, isOutput=False)
    sf_d = nc.declare_dram_parameter("sfield", [32, FDL], BF16, isOutput=False)
    ones_d = nc.declare_dram_parameter("ones", [32, RLEN], BF16, isOutput=False)
    lm_d = nc.declare_dram_parameter("lhsT_main", [51, 9 * 128], BF16, isOutput=False)
    ll_d = nc.declare_dram_parameter("lhsT_L", [128, 3 * 72], BF16, isOutput=False)
    ls_d = nc.declare_dram_parameter("lhsT_s", [72, 8], BF16, isOutput=False)
    lr_d = nc.declare_dram_parameter("lhsT_R", [8, 72], BF16, isOutput=False)
    la_d = nc.declare_dram_parameter("lhsT_A", [72, 3 * 128], BF16, isOutput=False)
    lp_d = nc.declare_dram_parameter("lhsT_P", [128, 2 * 32], BF16, isOutput=False)
    pb_d = nc.declare_dram_parameter("pbias", [64, 1], F32, isOutput=False)
    mr_d = nc.declare_dram_parameter("rowmask", [128, 2], F32, isOutput=False)
    out_d = nc.declare_dram_parameter("out", [64, 64 * 256], F32, isOutput=True)

    with tile.TileContext(nc) as tc:
      with tc.sbuf_pool(name="persist", bufs=1) as pp:
        FT = 2 + NF + 524
        lm = pp.tile([51, 9 * 128], BF16, name="lm")
        nc.sync.dma_start(out=lm[:], in_=lm_d.ap())
        ll = pp.tile([128, 3 * 72], BF16, name="ll")
        nc.sync.dma_start(out=ll[:], in_=ll_d.ap())
        ls = pp.tile([72, 8], BF16, name="ls")
        nc.sync.dma_start(out=ls[:], in_=ls_d.ap())
        lr = pp.tile([8, 72], BF16, name="lr")
        nc.sync.dma_start(out=lr[:], in_=lr_d.ap())
        la = pp.tile([72, 3 * 128], BF16, name="la")
        nc.sync.dma_start(out=la[:], in_=la_d.ap())
        lp = pp.tile([128, 2 * 32], BF16, name="lp")
        nc.sync.dma_start(out=lp[:], in_=lp_d.ap())
        pb = pp.tile([64, 1], F32, name="pb")
        nc.sync.dma_start(out=pb[:], in_=pb_d.ap())
        mr = pp.tile([128, 2], F32, name="mr")
        nc.sync.dma_start(out=mr[:], in_=mr_d.ap())


        # ---------------- main convs ----------------
        dp = tc.alloc_tile_pool(name="fdp", bufs=1, space="DRAM")
        fdram = dp.tile([128, FT], BF16, name="fdram")
        with tc.sbuf_pool(name="convp", bufs=1) as cp, \
             tc.sbuf_pool(name="stg", bufs=4) as sgp, \
             tc.psum_pool(name="cpsum", bufs=3) as cps:
            xin = cp.tile([51, 68 * WP + 2], BF16, name="xin")
            NB = 1032
            for i in range(17):
                nc.sync.dma_start(out=xin[:, 1 + i * NB:1 + (i + 1) * NB],
                                  in_=xin_d.ap()[:, i * NB:(i + 1) * NB])
            nchunks = (NF + CH - 1) // CH
            for c in range(nchunks):
                base = c * CH
                n = min(CH, NF - base)
                ps = cps.tile([128, CH], F32, name="cps", tag="cps")
                it = 0
                for dy in range(3):
                    for dx in range(3):
                        nc.tensor.matmul(
                            ps[:, 0:n],
                            lm[:, it * 128:(it + 1) * 128],
                            xin[:, base + dy * WP + dx: base + dy * WP + dx + n],
                            start=(it == 0), stop=(it == 8))
                        it += 1
                st = sgp.tile([128, CH], BF16, name="st", tag="st")
                nc.vector.tensor_copy(st[:, 0:n], ps[:, 0:n])
                # zero the padded columns (y==0 and y==257 of each field row)
                w = ((base + WP - 1) // WP) * WP - base
                while w < n:
                    nc.vector.memset(st[:, w:w + 1], 0.0)
                    if w + WP - 1 < n:
                        nc.vector.memset(st[:, w + WP - 1:w + WP], 0.0)
                    w += WP
                wl = ((base + WP - 1) // WP) * WP - base - 1   # col 257 of prev row
                if 0 <= wl < n:
                    nc.vector.memset(st[:, wl:wl + 1], 0.0)
                # mask out-of-image top/bottom field rows (row 0 / row 65)
                if base == 0:
                    nc.vector.tensor_scalar_mul(st[:, 0:WP], st[:, 0:WP], mr[:, 0:1])
                r65a, r65b = 65 * WP, 66 * WP
                lo = max(base, r65a); hi = min(base + n, r65b)
                if lo < hi:
                    nc.vector.tensor_scalar_mul(st[:, lo - base:hi - base],
                                                st[:, lo - base:hi - base], mr[:, 1:2])
                nc.gpsimd.dma_start(out=fdram[:, 2 + base:2 + base + n],
                                    in_=st[:, 0:n])

        # ---------------- attention ----------------
        with tc.sbuf_pool(name="attn", bufs=2) as ap_, \
             tc.sbuf_pool(name="attn1", bufs=1) as ap1, \
             tc.psum_pool(name="apsum", bufs=1) as aps, \
             tc.psum_pool(name="apsA", bufs=3) as apsA:
            q3 = pp.tile([128, RLEN], BF16, name="q3")
            k3p = pp.tile([128, RLEN], BF16, name="k3p")
            k3m = pp.tile([128, RLEN], BF16, name="k3m")
            v3p = pp.tile([128, RLEN], BF16, name="v3p")
            v3m = pp.tile([128, RLEN], BF16, name="v3m")
            for t in (k3p, k3m, v3p, v3m):
                nc.gpsimd.dma_start(out=t[96:128, :], in_=ones_d.ap())
            for blk in range(NBLK):
                gbase = blk * BR * WP
                nc.gpsimd.dma_start(
                    out=q3[:, 0:PF + RMARG],
                    in_=fdram[0:32, 2 + gbase:2 + gbase + PF + RMARG]
                        .rearrange("c (u f) -> u c f", u=1)
                        .broadcast_to([4, 32, PF + RMARG]))
                xblk = ap1.tile([64, PF], F32, name="xblk", tag="xblk")
                for bi in range(2):
                    k3 = k3p if bi == 0 else k3m
                    v3 = v3p if bi == 0 else v3m
                    ksrc = fdram[32:64] if bi == 0 else sf_d.ap()[0:32]
                    vsrc = fdram[96:128] if bi == 0 else fdram[64:96]
                    for dx in range(3):
                        off = 2 + gbase + dx - 1
                        nc.gpsimd.dma_start(
                            out=k3[32 * dx:32 * dx + 32, 0:PF + RMARG],
                            in_=ksrc[:, off:off + PF + RMARG])
                        nc.gpsimd.dma_start(
                            out=v3[32 * dx:32 * dx + 32, 0:PF + RMARG],
                            in_=vsrc[:, off:off + PF + RMARG])
                    pt = []
                    for dy in range(3):
                        p = ap1.tile([128, PF], BF16, name=f"p{dy}", tag=f"p{dy}")
                        nc.vector.tensor_tensor(
                            out=p[:], in0=q3[:, WP:WP + PF],
                            in1=k3[:, dy * WP:dy * WP + PF], op=ALU.mult)
                        pt.append(p)
                    for c in range(NCH):
                        cb = c * CH
                        lps = aps.tile([72, CH], F32, name="lps", tag="lps")
                        for dy in range(3):
                            nc.tensor.matmul(
                                lps[:], ll[:, dy * 72:(dy + 1) * 72],
                                pt[dy][:, cb:cb + CH],
                                start=(dy == 0), stop=(dy == 2))
                        e = ap_.tile([72, CH], BF16, name="e", tag="e")
                        nc.scalar.activation(e[:], lps[:], AF.Exp)
                        s0p = aps.tile([8, CH], F32, name="s0p", tag="s0p")
                        nc.tensor.matmul(s0p[:], ls[:], e[:], start=True, stop=True)
                        rr = ap_.tile([8, CH], BF16, name="rr", tag="rr")
                        with nc.allow_low_precision(reason="softmax recip"):
                            nc.vector.reciprocal(rr[:], s0p[:])
                        r72 = aps.tile([72, CH], F32, name="r72", tag="r72")
                        nc.tensor.matmul(r72[:], lr[:], rr[:], start=True, stop=True)
                        at = ap_.tile([72, CH], BF16, name="at", tag="at")
                        nc.vector.tensor_tensor(out=at[:], in0=e[:], in1=r72[:],
                                                op=ALU.mult)
                        us = None
                        for dy in range(3):
                            ax = apsA.tile([128, CH], F32, name="ax", tag="ax")
                            nc.tensor.matmul(ax[:], la[:, dy * 128:(dy + 1) * 128],
                                             at[:], start=True, stop=True)
                            u = ap_.tile([128, CH], BF16, name=f"u{dy}", tag=f"u{dy}")
                            nc.vector.tensor_tensor(
                                out=u[:], in0=ax[:],
                                in1=v3[:, dy * WP + cb:dy * WP + cb + CH],
                                op=ALU.mult)
                            if us is None:
                                us = u
                            else:
                                dst = ap_.tile([128, CH], BF16, name="usum",
                                               tag="usum")
                                nc.vector.tensor_tensor(out=dst[:], in0=us[:],
                                                        in1=u[:], op=ALU.add)
                                us = dst
                        xps = aps.tile([32, CH], F32, name="xps", tag="xps")
                        nc.tensor.matmul(xps[:], lp[:, bi * 32:(bi + 1) * 32],
                                         us[:], start=True, stop=True)
                        nc.scalar.activation(
                            xblk[bi * 32:(bi + 1) * 32, cb:cb + CH], xps[:],
                            AF.Identity, bias=pb[bi * 32:(bi + 1) * 32, :])
                nc.sync.dma_start(
                    out=out_d.ap()[:, blk * BR * 256:(blk + 1) * BR * 256],
                    in_=xblk[:, 0:PGRID].rearrange("p (r w) -> p r w", r=BR)[:, :, 1:257])
    if not nc.is_finalized():
        nc.finalize()
    _CACHE["nc"] = nc
    return nc


# ---------------------------------------------------------------- entry
def kernel(x, ms, lpan, pan, s, w_q, w_kpan, w_vpan, w_kvms, w_dep, b_dep,
           w_proj_pan, b_proj_pan, w_proj_ms, b_proj_ms):
    x, ms, lpan, pan = [np.asarray(t, np.float32) for t in (x, ms, lpan, pan)]
    s = np.asarray(s, np.float32)
    LL, Ls, LR, LA, LP, pbias = _attn_weights(
        np.asarray(w_dep, np.float32), np.asarray(b_dep, np.float32),
        np.asarray(w_proj_pan, np.float32), np.asarray(b_proj_pan, np.float32),
        np.asarray(w_proj_ms, np.float32), np.asarray(b_proj_ms, np.float32))
    bf = ml_dtypes.bfloat16
    common = {
        "ones": _np(np.ones((32, RLEN), bf)),
        "lhsT_L": _np(LL.transpose(1, 0, 2).reshape(128, -1).astype(bf)),
        "lhsT_s": _np(Ls.astype(bf)),
        "lhsT_R": _np(LR.astype(bf)),
        "lhsT_A": _np(LA.transpose(1, 0, 2).reshape(72, -1).astype(bf)),
        "lhsT_P": _np(LP.transpose(1, 0, 2).reshape(128, -1).astype(bf)),
        "pbias": _np(pbias.reshape(64, 1)),
    }
    in_maps = []
    for core in range(8):
        b, r0 = core // 4, (core % 4) * 64
        lm = _fold_main_weights(np.asarray(w_q, np.float32),
                                np.asarray(w_kvms, np.float32),
                                np.asarray(w_vpan, np.float32), float(s[b]))
        xinp = np.zeros((51, 68, WP), np.float32)
        lo, hi = max(0, r0 - 2), min(256, r0 + 66)
        sl = np.s_[lo:hi]
        o = lo - (r0 - 2)
        n = hi - lo
        xinp[0:32, o:o + n, 1:257] = x[b][:, sl]
        xinp[32:40, o:o + n, 1:257] = ms[b][:, sl]
        xinp[40, o:o + n, 1:257] = lpan[b, 0, sl]
        xinp[41:49, o:o + n, 1:257] = ms[b][:, sl]
        xinp[49, o:o + n, 1:257] = lpan[b, 0, sl]
        xinp[50, o:o + n, 1:257] = pan[b, 0, sl]
        sf = _host_sfield(x, ms, np.asarray(w_kvms, np.float32), b, r0)
        m = dict(common)
        rm = np.ones((128, 2), np.float32)
        if r0 == 0:
            rm[:, 0] = 0.0
        if r0 == 192:
            rm[:, 1] = 0.0
        m["rowmask"] = _np(rm)
        m["xin"] = _np(xinp.reshape(51, -1).astype(bf))
        sfp = np.zeros((32, 2 + NF + 524), np.float32)
        sfp[:, 2:2 + NF] = sf.reshape(32, -1)
        m["sfield"] = _np(sfp.astype(bf))
        m["lhsT_main"] = _np(lm.transpose(1, 0, 2).reshape(51, -1).astype(bf))
        in_maps.append(m)

    nc = _build_nc()
    _CACHE["in_maps"] = in_maps
    res = run_bass_kernel_spmd(nc, in_maps, core_ids=list(range(8)))
    x_pan = np.zeros((B, 32, H, W), np.float32)
    x_ms = np.zeros((B, 32, H, W), np.float32)
    for core in range(8):
        b, r0 = core // 4, (core % 4) * 64
        o = res.results[core]["out"].reshape(64, 64, 256)
        x_pan[b, :, r0:r0 + 64] = o[0:32]
        x_ms[b, :, r0:r0 + 64] = o[32:64]
    return (x_pan, x_ms)



# revision 20
# speedup vs baseline: 6925.8089x; 6925.8089x over previous
"""Trainium2 Bass kernel for nn_CMAAA_29274497089816 (sparse local attention).

Sharding: data-parallel B(2) x H-slab(4) over 8 cores. Each core handles one
batch sample and a 64-row output slab. Host prepares padded input slabs,
folded conv weights (cond/s and pan-lpan folds baked in), and the scrambled
k_ms "S" field (vectorized gemm conv); the chip runs the big convs and the
full neighborhood attention.

The compiled executable (jit of the bass custom call over an 8-core mesh)
is built once and cached; subsequent kernel() calls only re-run host prep,
upload fresh inputs, execute, and download the fp16 output.
"""
import sys, os
sys.path.insert(0, "/opt/trn_rl_repo")
import numpy as np
import ml_dtypes

import concourse.bass as bass
import concourse.bacc as bacc
import concourse.mybir as mybir
from concourse import tile

BF16 = mybir.dt.bfloat16
F32 = mybir.dt.float32
F16 = mybir.dt.float16
AF = mybir.ActivationFunctionType
ALU = mybir.AluOpType

DIM, HEADS, KA, MS_C, B, H, W = 32, 8, 3, 8, 2, 256, 256
HD, KK = 4, 9
SCALE = HD ** -0.5

NROW = 66            # field rows r0-1 .. r1+1
WP = 258             # padded width
NF = NROW * WP       # 17028 field pixels
NBLK = 4             # attention row-blocks per core
BR = 16              # out rows per block
PF = BR * WP         # 4128 product px per block (exact, no tail waste)
CHUNKS = [512] * 8 + [32]          # 8*512 + 32 = 4128
RMARG = 2 * WP + 2   # replica tile read margin
RLEN = PF + RMARG    # 4646 window elems
SFL = 2 + NF + 6     # sfield dram length (lead 2, tail 6)
bf = ml_dtypes.bfloat16


def _np(x):
    return np.ascontiguousarray(x)


# ---------------------------------------------------------------- host prep
def _fold_main_weights(w_q, w_kvms, w_vpan, sb):
    """lhsT_main[9, 42, 128]: rows [x32, ms8, lpan1, pan1],
    cols [q(scaled)32, k_ms32, v_ms32, v_pan32]."""
    Ls = np.zeros((3, 3, 42, 128), np.float32)
    for dy in range(3):
        for dx in range(3):
            L = Ls[dy, dx]
            Wq = w_q[:, :, dy, dx]
            L[0:32, 0:32] = Wq[:, 0:32].T * SCALE
            L[32:40, 0:32] = Wq[:, 32:40].T * SCALE * sb
            L[40, 0:32] = Wq[:, 32:40].sum(1) * SCALE * (1.0 - sb)
            Wk = w_kvms[:, :, dy, dx]
            L[0:32, 32:64] = Wk[0:32, 0:32].T
            L[32:40, 32:64] = Wk[0:32, 32:40].T
            L[0:32, 64:96] = Wk[32:64, 0:32].T
            L[32:40, 64:96] = Wk[32:64, 32:40].T
            Wv = w_vpan[:, :, dy, dx]
            L[0:32, 96:128] = Wv[:, 0:32].T
            L[40, 96:128] = Wv[:, 32] - Wv[:, 34]
            L[41, 96:128] = Wv[:, 33] + Wv[:, 34]
    # pack (dx, ch) into the 126-row contraction dim per dy
    return Ls.transpose(0, 1, 2, 3).reshape(3, 126, 128)


def _attn_weights(w_dep, b_dep, w_proj_pan, b_proj_pan, w_proj_ms, b_proj_ms):
    Wd = np.zeros((4, 9, 9), np.float32)          # [d, t, j]
    for d in range(4):
        for j in range(9):
            Wd[d, :, j] = w_dep[d * 9 + j, 0].reshape(9)
    bd = b_dep.reshape(4, 9)                      # [d, j]

    # logits MM weights: lhsT_L[dy] [128, 72]; rows (dx,h,d) 0:96, q-rows 96:128
    L_L = np.zeros((3, 128, 72), np.float32)
    for dy in range(3):
        for dx in range(3):
            t = dy * 3 + dx
            for h in range(8):
                for d in range(4):
                    for j in range(9):
                        L_L[dy, dx * 32 + h * 4 + d, h * 9 + j] = Wd[d, t, j]
    for h in range(8):
        for d in range(4):
            for j in range(9):
                L_L[1, 96 + h * 4 + d, h * 9 + j] = bd[d, j]   # qb bias term

    # s0 sum MM: lhsT_s [72, 8]
    L_s = np.zeros((72, 8), np.float32)
    for h in range(8):
        L_s[h * 9:(h + 1) * 9, h] = 1.0
    # R72 broadcast MM: lhsT_R [8, 72]
    L_R = np.zeros((8, 72), np.float32)
    for h in range(8):
        L_R[h, h * 9:(h + 1) * 9] = 1.0
    # A MMs: lhsT_A[dy] [72, 128]: cols (dx,h,d) 0:96; dy==1 cols 96:128 = ba
    L_A = np.zeros((3, 72, 128), np.float32)
    for dy in range(3):
        for dx in range(3):
            t = dy * 3 + dx
            for h in range(8):
                for d in range(4):
                    for j in range(9):
                        L_A[dy, h * 9 + j, dx * 32 + h * 4 + d] = Wd[d, t, j]
    for h in range(8):
        for d in range(4):
            for j in range(9):
                L_A[1, h * 9 + j, 96 + h * 4 + d] = bd[d, j]
    # proj: lhsT_P[2, 128, 32]: rows (dx,h,d) = Wp.T replicated; rows 96:128 Wp.T
    L_P = np.zeros((2, 128, 32), np.float32)
    for bi, wp in enumerate([w_proj_pan, w_proj_ms]):
        wt = wp[:, :, 0, 0].T                     # [32in(h,d), 32out]
        for dx in range(3):
            L_P[bi, dx * 32:(dx + 1) * 32] = wt
        L_P[bi, 96:128] = wt
    pbias = np.stack([b_proj_pan, b_proj_ms]).reshape(2, 32, 1).astype(np.float32)
    return L_L, L_s, L_R, L_A, L_P, pbias


def _host_kimg(x, ms, w_kvms, b):
    """k_ms conv over the full image for batch b, via 9 accumulated gemms
    on the flat padded grid. Returns kimg[32, 256, 258] (cols 1..256 real)."""
    xin = np.concatenate([x[b], ms[b]], 0)            # (40, 256, 256)
    xp = np.zeros((40, 259, 258), np.float32)         # extra zero row for tap OOB
    xp[:, 1:257, 1:257] = xin
    XF = xp.reshape(40, -1)
    Wk = w_kvms[0:32]                                 # (32, 40, 3, 3)
    N = 256 * 258                                     # grid rows 0..255 full width
    acc = np.zeros((32, N), np.float32)
    for dy in range(3):
        for dx in range(3):
            acc += np.ascontiguousarray(Wk[:, :, dy, dx]) @ \
                XF[:, dy * 258 + dx: dy * 258 + dx + N]
    return acc.reshape(32, 256, 258)                  # [:, y, 1+x] = kimg(y, x)


def _host_sfield(kimg, r0):
    """Scrambled k_ms field [32, NROW, WP] for slab r0, from kimg."""
    X = np.arange(r0 - 1, r0 + 65)                    # 66 field rows
    valid = (X >= 0) & (X < 256)
    Xc = np.clip(X, 0, 255)
    cp = np.arange(32)                                # c' = hh*4+dp
    hh_ = cp // 4; dp_ = cp % 4
    ch = hh_[:, None] * 4 + (Xc[None, :] % 4)         # (32, 66)
    col = dp_[:, None] * 64 + (Xc[None, :] // 4)      # (32, 66)
    S = np.zeros((32, NROW, WP), np.float32)
    # kimg[ch, y, col+1] over y -> S[c', gi, 1+y]
    S[:, :, 1:257] = kimg[ch[:, :, None],
                          np.arange(256)[None, None, :],
                          (col + 1)[:, :, None]]
    S[:, ~valid, :] = 0.0
    return S


# ---------------------------------------------------------------- bass build
_CACHE = {}


def _build_nc(kreps=1):
    key = f"nc{kreps}"
    if key in _CACHE:
        return _CACHE[key]
    nc = bacc.Bacc(None, target_bir_lowering=False)
    FDL = 2 + NF + 524
    xin_d = nc.declare_dram_parameter("xin", [42, 68 * WP], BF16, isOutput=False)
    sf_d = nc.declare_dram_parameter("sfield", [32, FDL], BF16, isOutput=False)
    lm_d = nc.declare_dram_parameter("lhsT_main", [126, 3 * 128], BF16, isOutput=False)
    ll_d = nc.declare_dram_parameter("lhsT_L", [128, 3 * 72], BF16, isOutput=False)
    ls_d = nc.declare_dram_parameter("lhsT_s", [72, 8], BF16, isOutput=False)
    lr_d = nc.declare_dram_parameter("lhsT_R", [8, 72], BF16, isOutput=False)
    la_d = nc.declare_dram_parameter("lhsT_A", [72, 3 * 128], BF16, isOutput=False)
    lp_d = nc.declare_dram_parameter("lhsT_P", [128, 2 * 32], BF16, isOutput=False)
    pb_d = nc.declare_dram_parameter("pbias", [64, 1], F32, isOutput=False)
    mr_d = nc.declare_dram_parameter("rowmask", [128, 2], F32, isOutput=False)
    out_d = nc.declare_dram_parameter("out", [64, 64 * 256], F16, isOutput=True)

    with tile.TileContext(nc) as tc:
      with tc.sbuf_pool(name="persist", bufs=1) as pp:
        FT = FDL
        lm = pp.tile([126, 3 * 128], BF16, name="lm")
        nc.sync.dma_start(out=lm[:], in_=lm_d.ap())
        ll = pp.tile([128, 3 * 72], BF16, name="ll")
        nc.sync.dma_start(out=ll[:], in_=ll_d.ap())
        ls = pp.tile([72, 8], BF16, name="ls")
        nc.sync.dma_start(out=ls[:], in_=ls_d.ap())
        lr = pp.tile([8, 72], BF16, name="lr")
        nc.sync.dma_start(out=lr[:], in_=lr_d.ap())
        la = pp.tile([72, 3 * 128], BF16, name="la")
        nc.sync.dma_start(out=la[:], in_=la_d.ap())
        lp = pp.tile([128, 2 * 32], BF16, name="lp")
        nc.sync.dma_start(out=lp[:], in_=lp_d.ap())
        pb = pp.tile([64, 1], F32, name="pb")
        nc.sync.dma_start(out=pb[:], in_=pb_d.ap())
        mr = pp.tile([128, 2], F32, name="mr")
        nc.sync.dma_start(out=mr[:], in_=mr_d.ap())

        # persistent replica tiles; rows 96:128 are constant 1.0 (bias path)
        q3 = pp.tile([128, RLEN], BF16, name="q3")
        k3p = pp.tile([128, RLEN], BF16, name="k3p")
        k3m = pp.tile([128, RLEN], BF16, name="k3m")
        v3p = pp.tile([128, RLEN], BF16, name="v3p")
        v3m = pp.tile([128, RLEN], BF16, name="v3m")
        for t in (k3p, k3m, v3p, v3m):
            nc.gpsimd.memset(t[96:128, :], 1.0)

        # ---------------- main convs ----------------
        dp = tc.alloc_tile_pool(name="fdp", bufs=1, space="DRAM")
        fdram = dp.tile([128, FT], BF16, name="fdram")
        for rep in range(kreps):
          with tc.sbuf_pool(name=f"convp{rep}", bufs=1) as cp, \
             tc.sbuf_pool(name=f"stg{rep}", bufs=4) as sgp, \
             tc.psum_pool(name=f"cpsum{rep}", bufs=3) as cps:
            XW = 68 * WP + 4
            xin = cp.tile([126, XW], BF16, name="xin")
            ND = 68 * WP          # 17544 dram cols
            zt = cp.tile([128, 8], BF16, name="zt")
            nc.vector.memset(zt[:], 0.0)
            nc.sync.dma_start(out=xin[0:42, 0:1], in_=zt[0:42, 0:1])
            nc.sync.dma_start(out=xin[0:42, 1 + ND:XW], in_=zt[0:42, 0:XW - 1 - ND])
            nc.sync.dma_start(out=xin[42:84, ND:XW], in_=zt[0:42, 0:XW - ND])
            nc.sync.dma_start(out=xin[84:126, ND - 1:XW], in_=zt[0:42, 0:XW - ND + 1])
            nc.gpsimd.dma_start(out=fdram[:, 0:2], in_=zt[:, 0:2])
            nc.gpsimd.dma_start(out=fdram[:, 2 + NF:2 + NF + 8], in_=zt[:])
            NB = 1032
            for i in range(17):
                sl = np.s_[i * NB:(i + 1) * NB]
                lo, hi = i * NB, (i + 1) * NB
                nc.sync.dma_start(out=xin[0:42, 1 + lo:1 + hi],
                                  in_=xin_d.ap()[:, lo:hi])
                nc.scalar.dma_start(out=xin[42:84, lo:hi],
                                    in_=xin_d.ap()[:, lo:hi])
                nc.sync.dma_start(out=xin[84:126, max(0, lo - 1):hi - 1],
                                  in_=xin_d.ap()[:, max(1, lo):hi])
            CH = 512
            nchunks = (NF + CH - 1) // CH
            for c in range(nchunks):
                base = c * CH
                n = min(CH, NF - base)
                ps = cps.tile([128, CH], F32, name="cps", tag="cps")
                for dy in range(3):
                    nc.tensor.matmul(
                        ps[:, 0:n],
                        lm[:, dy * 128:(dy + 1) * 128],
                        xin[:, base + dy * WP: base + dy * WP + n],
                        start=(dy == 0), stop=(dy == 2))
                st = sgp.tile([128, CH], BF16, name="st", tag="st")
                nc.scalar.activation(st[:, 0:n], ps[:, 0:n], AF.Identity)
                # zero the padded columns (y==0 and y==257 of each field row)
                w = ((base + WP - 1) // WP) * WP - base
                while w < n:
                    nc.vector.memset(st[:, w:w + 1], 0.0)
                    if w + WP - 1 < n:
                        nc.vector.memset(st[:, w + WP - 1:w + WP], 0.0)
                    w += WP
                wl = ((base + WP - 1) // WP) * WP - base - 1   # col 257 of prev row
                if 0 <= wl < n:
                    nc.vector.memset(st[:, wl:wl + 1], 0.0)
                # mask out-of-image top/bottom field rows (row 0 / row 65)
                if base == 0:
                    nc.vector.tensor_scalar_mul(st[:, 0:WP], st[:, 0:WP], mr[:, 0:1])
                r65a, r65b = 65 * WP, 66 * WP
                lo = max(base, r65a); hi = min(base + n, r65b)
                if lo < hi:
                    nc.vector.tensor_scalar_mul(st[:, lo - base:hi - base],
                                                st[:, lo - base:hi - base], mr[:, 1:2])
                nc.gpsimd.dma_start(out=fdram[:, 2 + base:2 + base + n],
                                    in_=st[:, 0:n])

          # ---------------- attention ----------------
          with tc.sbuf_pool(name=f"attn{rep}", bufs=2) as ap_, \
             tc.sbuf_pool(name=f"attn1{rep}", bufs=1) as ap1, \
             tc.psum_pool(name=f"apsum{rep}", bufs=1) as aps, \
             tc.psum_pool(name=f"apsA{rep}", bufs=3) as apsA:
            for blk in range(NBLK):
                gbase = blk * BR * WP
                nc.gpsimd.dma_start(
                    out=q3[:, 0:RLEN],
                    in_=fdram[0:32, 2 + gbase:2 + gbase + RLEN]
                        .rearrange("c (u f) -> u c f", u=1)
                        .broadcast_to([4, 32, RLEN]))
                xblk = ap1.tile([64, PF], F16, name="xblk", tag="xblk")
                for bi in range(2):
                    k3 = k3p if bi == 0 else k3m
                    v3 = v3p if bi == 0 else v3m
                    ksrc = fdram[32:64] if bi == 0 else sf_d.ap()[0:32]
                    vsrc = fdram[96:128] if bi == 0 else fdram[64:96]
                    for dx in range(3):
                        off = 2 + gbase + dx - 1
                        nc.scalar.dma_start(
                            out=k3[32 * dx:32 * dx + 32, 0:RLEN],
                            in_=ksrc[:, off:off + RLEN])
                        nc.sync.dma_start(
                            out=v3[32 * dx:32 * dx + 32, 0:RLEN],
                            in_=vsrc[:, off:off + RLEN])
                    pt = []
                    for dy in range(3):
                        p = ap1.tile([128, PF], BF16, name=f"p{dy}", tag=f"p{dy}")
                        eng = nc.gpsimd if dy == 0 else nc.vector
                        eng.tensor_tensor(
                            out=p[:], in0=q3[:, WP:WP + PF],
                            in1=k3[:, dy * WP:dy * WP + PF], op=ALU.mult)
                        pt.append(p)
                    cb = 0
                    for n in CHUNKS:
                        lps = aps.tile([72, 512], F32, name="lps", tag="lps")
                        for dy in range(3):
                            nc.tensor.matmul(
                                lps[:, 0:n], ll[:, dy * 72:(dy + 1) * 72],
                                pt[dy][:, cb:cb + n],
                                start=(dy == 0), stop=(dy == 2))
                        e = ap_.tile([72, 512], BF16, name="e", tag="e")
                        nc.scalar.activation(e[:, 0:n], lps[:, 0:n], AF.Exp)
                        s0p = aps.tile([8, 512], F32, name="s0p", tag="s0p")
                        nc.tensor.matmul(s0p[:, 0:n], ls[:], e[:, 0:n],
                                         start=True, stop=True)
                        rr = ap_.tile([8, 512], BF16, name="rr", tag="rr")
                        with nc.allow_low_precision(reason="softmax recip"):
                            nc.vector.reciprocal(rr[:, 0:n], s0p[:, 0:n])
                        r72 = aps.tile([72, 512], F32, name="r72", tag="r72")
                        nc.tensor.matmul(r72[:, 0:n], lr[:], rr[:, 0:n],
                                         start=True, stop=True)
                        at = ap_.tile([72, 512], BF16, name="at", tag="at")
                        nc.vector.tensor_tensor(out=at[:, 0:n], in0=e[:, 0:n],
                                                in1=r72[:, 0:n], op=ALU.mult)
                        xps = aps.tile([32, 512], F32, name="xps", tag="xps")
                        for dy in range(3):
                            ax = apsA.tile([128, 512], F32, name="ax", tag="ax")
                            nc.tensor.matmul(ax[:, 0:n],
                                             la[:, dy * 128:(dy + 1) * 128],
                                             at[:, 0:n], start=True, stop=True)
                            u = ap_.tile([128, 512], BF16, name=f"u{dy}",
                                         tag=f"u{dy}")
                            nc.vector.tensor_tensor(
                                out=u[:, 0:n], in0=ax[:, 0:n],
                                in1=v3[:, dy * WP + cb:dy * WP + cb + n],
                                op=ALU.mult)
                            nc.tensor.matmul(xps[:, 0:n],
                                             lp[:, bi * 32:(bi + 1) * 32],
                                             u[:, 0:n],
                                             start=(dy == 0), stop=(dy == 2))
                        nc.scalar.activation(
                            xblk[bi * 32:(bi + 1) * 32, cb:cb + n], xps[:, 0:n],
                            AF.Identity, bias=pb[bi * 32:(bi + 1) * 32, :])
                        cb += n
                nc.sync.dma_start(
                    out=out_d.ap()[:, blk * BR * 256:(blk + 1) * BR * 256],
                    in_=xblk[:, 0:PF].rearrange("p (r w) -> p r w", r=BR)[:, :, 1:257])
    if not nc.is_finalized():
        nc.finalize()
    _CACHE["nc"] = nc
    return nc


# ---------------------------------------------------------------- jit runner
def _get_runner(kreps=1):
    rkey = f"runner{kreps}"
    if rkey in _CACHE:
        return _CACHE[rkey]
    import jax
    from jax.sharding import Mesh, PartitionSpec, NamedSharding
    from jax.experimental.shard_map import shard_map
    from concourse.bass2jax import _bass_exec_p, install_neuronx_cc_hook, \
        partition_id_tensor

    nc = _build_nc(kreps)
    install_neuronx_cc_hook()
    partition_name = nc.partition_id_tensor.name if nc.partition_id_tensor else None
    in_names, out_names, out_avals, zero_outs = [], [], [], []
    for alloc in nc.m.functions[0].allocations:
        if not isinstance(alloc, mybir.MemoryLocationSet):
            continue
        name = alloc.memorylocations[0].name
        if alloc.kind == "ExternalInput":
            if name != partition_name:
                in_names.append(name)
        elif alloc.kind == "ExternalOutput":
            out_names.append(name)
            shape = tuple(alloc.tensor_shape)
            dtype = mybir.dt.np(alloc.dtype)
            out_avals.append(jax.core.ShapedArray(shape, dtype))
            zero_outs.append(np.zeros(shape, dtype))
    n_params = len(in_names)
    all_in_names = in_names + out_names + \
        ([partition_name] if partition_name else [])

    def _body(*args):
        operands = list(args)
        if partition_name is not None:
            operands.append(partition_id_tensor())
        outs = _bass_exec_p.bind(
            *operands, out_avals=tuple(out_avals),
            in_names=tuple(all_in_names), out_names=tuple(out_names),
            lowering_input_output_aliases=(), sim_require_finite=True,
            sim_require_nnan=True, nc=nc)
        return tuple(outs)

    devices = jax.devices()[:8]
    mesh = Mesh(np.asarray(devices), ("core",))
    spec = PartitionSpec("core")
    in_specs = (spec,) * (n_params + len(out_names))
    out_specs = (spec,) * len(out_names)
    fn = jax.jit(shard_map(_body, mesh=mesh, in_specs=in_specs,
                           out_specs=out_specs, check_rep=False))
    sharding = NamedSharding(mesh, spec)
    # outputs are fully written by the kernel: keep the zero operand resident
    zeros_dev = [jax.device_put(np.concatenate([z] * 8, axis=0), sharding)
                 for z in zero_outs]
    runner = {"fn": fn, "in_names": in_names, "out_names": out_names,
              "zeros_dev": zeros_dev, "sharding": sharding, "jax": jax,
              "nc": nc, "body": _body, "mesh": mesh, "n_params": n_params,
              "out_avals": out_avals}
    _CACHE[rkey] = runner
    return runner


def _prep_in_maps(x, ms, lpan, pan, s, w_q, w_kpan, w_vpan, w_kvms, w_dep,
                  b_dep, w_proj_pan, b_proj_pan, w_proj_ms, b_proj_ms):
    x, ms, lpan, pan = [np.asarray(t, np.float32) for t in (x, ms, lpan, pan)]
    s = np.asarray(s, np.float32)
    LL, Ls, LR, LA, LP, pbias = _attn_weights(
        np.asarray(w_dep, np.float32), np.asarray(b_dep, np.float32),
        np.asarray(w_proj_pan, np.float32), np.asarray(b_proj_pan, np.float32),
        np.asarray(w_proj_ms, np.float32), np.asarray(b_proj_ms, np.float32))
    common = {
        "lhsT_L": _np(LL.transpose(1, 0, 2).reshape(128, -1).astype(bf)),
        "lhsT_s": _np(Ls.astype(bf)),
        "lhsT_R": _np(LR.astype(bf)),
        "lhsT_A": _np(LA.transpose(1, 0, 2).reshape(72, -1).astype(bf)),
        "lhsT_P": _np(LP.transpose(1, 0, 2).reshape(128, -1).astype(bf)),
        "pbias": _np(pbias.reshape(64, 1)),
    }
    w_kvms32 = np.asarray(w_kvms, np.float32)
    # per-batch: padded input stack (bf16 once) and full k_ms image
    xstack, kimgs, lms = [], [], []
    for b in range(B):
        xs = np.zeros((42, 260, WP), np.float32)   # rows -2..257, cols pad 1
        xs[0:32, 2:258, 1:257] = x[b]
        xs[32:40, 2:258, 1:257] = ms[b]
        xs[40, 2:258, 1:257] = lpan[b, 0]
        xs[41, 2:258, 1:257] = pan[b, 0]
        xstack.append(xs.astype(bf))
        kimgs.append(_host_kimg(x, ms, w_kvms32, b))
        lms.append(_fold_main_weights(
            np.asarray(w_q, np.float32), w_kvms32,
            np.asarray(w_vpan, np.float32), float(s[b])))
    in_maps = []
    for core in range(8):
        b, r0 = core // 4, (core % 4) * 64
        m = dict(common)
        rm = np.ones((128, 2), np.float32)
        if r0 == 0:
            rm[:, 0] = 0.0
        if r0 == 192:
            rm[:, 1] = 0.0
        m["rowmask"] = _np(rm)
        m["xin"] = _np(xstack[b][:, r0:r0 + 68, :].reshape(42, -1))
        sf = _host_sfield(kimgs[b], r0)
        sfp = np.zeros((32, 2 + NF + 524), bf)
        sfp[:, 2:2 + NF] = sf.reshape(32, -1).astype(bf)
        m["sfield"] = sfp
        m["lhsT_main"] = _np(lms[b].transpose(1, 0, 2).reshape(126, -1).astype(bf))
        in_maps.append(m)
    return in_maps


def _run(in_maps):
    r = _get_runner()
    jax = r["jax"]
    gl = [np.concatenate([m[name] for m in in_maps], axis=0)
          for name in r["in_names"]]
    gput = [jax.device_put(g, r["sharding"]) for g in gl]
    outs = r["fn"](*gput, *r["zeros_dev"])
    return np.asarray(outs[0])


# ---------------------------------------------------------------- entry
def kernel(x, ms, lpan, pan, s, w_q, w_kpan, w_vpan, w_kvms, w_dep, b_dep,
           w_proj_pan, b_proj_pan, w_proj_ms, b_proj_ms):
    in_maps = _prep_in_maps(x, ms, lpan, pan, s, w_q, w_kpan, w_vpan, w_kvms,
                            w_dep, b_dep, w_proj_pan, b_proj_pan, w_proj_ms,
                            b_proj_ms)
    _CACHE["in_maps"] = in_maps
    res = _run(in_maps)                     # [8*64, 64*256] f16
    res = res.reshape(8, 64, 64, 256).astype(np.float32)
    x_pan = np.zeros((B, 32, H, W), np.float32)
    x_ms = np.zeros((B, 32, H, W), np.float32)
    for core in range(8):
        b, r0 = core // 4, (core % 4) * 64
        x_pan[b, :, r0:r0 + 64] = res[core, 0:32]
        x_ms[b, :, r0:r0 + 64] = res[core, 32:64]
    return (x_pan, x_ms)
